# revision 30
# baseline (speedup 1.0000x reference)
"""AffinityNet Trainium2 kernel: 8-core data-parallel (1 batch per core).

Per-core pipeline (batch b), v2:
  f2 = elu(w2 @ f2_in[b]) + 1     [64, 64, 64]   fp8 DoubleRow matmul
  f2r = bilinear_resize(f2)       [64, 32, 32]   (4-tap separable, DVE)
  f3 = elu(w3 @ f3_in[b]) + 1     [128, 32, 32]  bf16
  f4 = elu(w4 @ f4_in[b]) + 1     [320, 32, 32]  bf16
  x  = elu(w9p @ xcat - rowsum(w9p)) + 1         [512, 32, 32]
  aff[p, i] = exp(-mean_c |x[c, to(p,i)] - x[c, from(i)]|)   [34, 672]

All elu's store elu+1 = min(exp(z),1) + relu(z) (2 ACT passes, then a DVE
tensor_scalar min + tensor_tensor add); the +1 shift cancels in the affinity
differences and S-corrections, and is removed before conv9 via a per-channel
bias b9 = -rowsum(w9p).

Affinity uses |a-b| = 2 max(a,b) - a - b: DVE computes per-(group, ctile)
maxes of shifted strided views (tensor_tensor runs at 2 elem/cycle/lane); PE
sums channels with an M=1 ones matmul into one PSUM row per slot, 4 slots
per PSUM bank at partition rows {0,32,64,96} (tile_position packing). The
-(S_from+S_to)/2 corrections are K=128 matmuls over s128 = -S/256 replicated
on all partitions (hi rows 0:64 + bf16-residual rows 64:128, recovering
~fp32 accuracy) using the SAME ones stationary as the sums — zero stationary
switches across all 408 affinity matmuls keeps the PE at full clock. One
batched ACT exp extracts each bank; the DMA gathers the 4 valid rows. Host
only shards/reassembles and un-scrambles the slot layout.
"""

import sys

for _p in ("/opt/trn_rl_repo", "/root/.axon_site", "/root/.axon_site/_ro/pypackages"):
    if _p not in sys.path:
        sys.path.append(_p)

import numpy as np
import ml_dtypes

BF16 = ml_dtypes.bfloat16
FP8 = ml_dtypes.float8_e4m3fn

# ---------------- problem constants (hardcoded from the spec) ----------------
B = 8
N_CORES = 8
CH = 28             # cropped height  (h range of "from" positions)
CW = 24             # cropped width   (w range of "from" positions)
NPOS = CH * CW      # 672
W2SCALE = 64.0      # fp8 weight scaling for conv2

# offset list in the reference order (dy, dx)
OFFSETS = [(0, x) for x in range(1, 5)] + [
    (y, x) for y in range(1, 5) for x in range(-4, 5) if x * x + y * y < 25
]
assert len(OFFSETS) == 34

# groups: same dy, same-parity dx run (step 2) -> one DVE max per (group, ct)
GROUPS = [
    (0, [2, 4]),
    (0, [1, 3]),
    (1, [-4, -2, 0, 2, 4]),
    (1, [-3, -1, 1, 3]),
    (2, [-4, -2, 0, 2, 4]),
    (2, [-3, -1, 1, 3]),
    (3, [-2, 0, 2]),
    (3, [-3, -1, 1, 3]),
    (4, [-2, 0, 2]),
    (4, [-1, 1]),
]
assert sorted(set((dy, dx) for dy, dxs in GROUPS for dx in dxs)) == sorted(OFFSETS)

CHUNK = 336  # half of NPOS: slot = (group, dxi, h-half)

# slot list: (group, start col within group, size)
_SLOTS = []
for _g, (_dy, _dxs) in enumerate(GROUPS):
    _G = len(_dxs) * NPOS
    _c0 = 0
    while _c0 < _G:
        _cs = min(CHUNK, _G - _c0)
        _SLOTS.append((_g, _c0, _cs))
        _c0 += _cs
N_SLOTS = len(_SLOTS)                 # 68
N_BANKS = N_SLOTS // 4                # 17
assert N_SLOTS % 4 == 0


def _build_unscramble():
    """aff[p, i] = out_flat[slot*CHUNK + col]; build (slot, col) index maps."""
    off_index = {od: i for i, od in enumerate(OFFSETS)}
    slot_of = np.zeros((34, NPOS), np.int64)
    col_of = np.zeros((34, NPOS), np.int64)
    slot_base = 0
    for g, (dy, dxs) in enumerate(GROUPS):
        G = len(dxs) * NPOS
        nchunks = (G + CHUNK - 1) // CHUNK
        for k, dx in enumerate(dxs):
            p = off_index[(dy, dx)]
            gcols = k * NPOS + np.arange(NPOS)
            slot_of[p, :] = slot_base + gcols // CHUNK
            col_of[p, :] = gcols % CHUNK
        slot_base += nchunks
    assert slot_base == N_SLOTS
    return slot_of, col_of


_SLOT_OF, _COL_OF = _build_unscramble()
_FLAT_IDX = _SLOT_OF * CHUNK + _COL_OF

# bilinear 64->32 resize taps (jax.image.resize, triangle kernel, antialias):
# interior: out[j] = .125 in[2j-1] + .375 in[2j] + .375 in[2j+1] + .125 in[2j+2]
# out[0] = (3 in[0] + 3 in[1] + in[2]) / 7 ; out[31] = (in[61] + 3 in[62] + 3 in[63]) / 7
W_IN = 0.375
W_OUT = 0.125
B3 = 3.0 / 7.0
B1 = 1.0 / 7.0

# xcat channel layout = [f3(128) | f4 m0(128) | f4 m1(128) | f2r(64)+f4 m2(64)];
# original concat order = [f2r 0:64 | f3 64:192 | f4 192:512]
XCAT_PERM = (
    list(range(64, 192))
    + list(range(192, 320))
    + list(range(320, 448))
    + list(range(0, 64))
    + list(range(448, 512))
)


# ------------------------------- bass kernel ---------------------------------

def _fview(t_ap, off, dims):
    """Strided free-dim view of a tile AP (partition dim preserved)."""
    import concourse.bass as bass

    pd = list(t_ap.ap)[0]
    return bass.AP(
        tensor=t_ap.tensor,
        offset=t_ap.offset + off,
        ap=[list(pd)] + [list(d) for d in dims],
    )


def _prow_view(t_ap, row_stride, nrows, ncols):
    """Partition-strided view (for the output DMA row gather)."""
    import concourse.bass as bass

    pd = list(t_ap.ap)[0]
    return bass.AP(
        tensor=t_ap.tensor,
        offset=t_ap.offset,
        ap=[[row_stride * pd[0], nrows], [1, ncols]],
    )


def build_nc():
    """Build + compile the per-core Bass program."""
    from contextlib import ExitStack

    import concourse.tile as tile
    from concourse import bacc, mybir

    dt = mybir.dt
    ALU = mybir.AluOpType
    ACTF = mybir.ActivationFunctionType
    DR = mybir.MatmulPerfMode.DoubleRow

    nc = bacc.Bacc("TRN2", target_bir_lowering=False, debug=False)

    # f2: fp8, n-block-major: [p, nb*2048 + kt*512 + c], nb in 0..7, kt in 0..3
    f2d = nc.dram_tensor("f2", [128, 8 * 2048], dt.float8e4, kind="ExternalInput").ap()
    # f3: bf16, n-block-major: [p, nb*4096 + kt*512 + c], nb in 0..1, kt in 0..7
    f3d = nc.dram_tensor("f3", [128, 2 * 4096], dt.bfloat16, kind="ExternalInput").ap()
    # f4: bf16, n-block-major: [p, nb*8192 + kt*512 + c], nb in 0..1, kt in 0..15
    f4d = nc.dram_tensor("f4", [128, 2 * 8192], dt.bfloat16, kind="ExternalInput").ap()
    # w2: fp8 (x64), [p, kp*128 + j*64 + m] (ktile-pair layout for DoubleRow)
    w2d = nc.dram_tensor("w2t", [128, 4 * 64], dt.float8e4, kind="ExternalInput").ap()
    w3d = nc.dram_tensor("w3t", [128, 8 * 128], dt.bfloat16, kind="ExternalInput").ap()
    w4d = nc.dram_tensor("w4t", [128, 16 * 320], dt.bfloat16, kind="ExternalInput").ap()
    w9d = nc.dram_tensor("w9t", [128, 4 * 512], dt.bfloat16, kind="ExternalInput").ap()
    b9d = nc.dram_tensor("b9", [128, 4], dt.float32, kind="ExternalInput").ap()
    outd = nc.dram_tensor(
        "aff", [N_SLOTS, CHUNK], dt.float32, kind="ExternalOutput"
    ).ap()

    with tile.TileContext(nc) as tc, ExitStack() as ctx:
        wpool = ctx.enter_context(tc.tile_pool(name="w", bufs=1))
        fpool = ctx.enter_context(tc.tile_pool(name="fin", bufs=1))
        xpool = ctx.enter_context(tc.tile_pool(name="x", bufs=1))
        pspool = ctx.enter_context(tc.tile_pool(name="ps", bufs=3, space="PSUM"))
        psb_pool = ctx.enter_context(tc.tile_pool(name="psb", bufs=4, space="PSUM"))
        tpool = ctx.enter_context(tc.tile_pool(name="tmp", bufs=4))
        mpool = ctx.enter_context(tc.tile_pool(name="mx", bufs=8))
        opool = ctx.enter_context(tc.tile_pool(name="out", bufs=3))

        # ---- weights + constants ----
        w2sb = wpool.tile([128, 4 * 64], dt.float8e4, tag="w2")
        nc.sync.dma_start(w2sb[:], w2d[:])
        ones = wpool.tile([128, 4], dt.bfloat16, tag="ones")
        nc.vector.memset(ones[:], 1.0)
        ones128 = wpool.tile([128, 128], dt.bfloat16, tag="ones128")
        nc.vector.memset(ones128[:], 1.0)

        xcat = xpool.tile([128, 4 * 1024], dt.bfloat16, tag="xcat")
        xbuf = xpool.tile([128, 4 * 1024], dt.bfloat16, tag="xbuf")
        xodd = xpool.tile([128, 4 * 1024], dt.bfloat16, tag="xodd")

        def elu1(ps_t, pb, psz, dst_ap, scale=1.0, bias=0.0):
            """dst = elu(scale*ps + bias) + 1 = min(exp(z),1) + relu(z)."""
            ps_ap = ps_t[pb : pb + psz, :]
            n = ps_ap.shape[-1]
            e = tpool.tile([128, 512], dt.bfloat16, tag="ee", name="ee")
            r = tpool.tile([128, 512], dt.bfloat16, tag="er", name="er")
            nc.scalar.activation(
                e[pb : pb + psz, 0:n], ps_ap, ACTF.Exp, scale=scale, bias=bias
            )
            nc.scalar.activation(
                r[pb : pb + psz, 0:n], ps_ap, ACTF.Relu, scale=scale, bias=bias
            )
            t = tpool.tile([128, 512], dt.bfloat16, tag="et", name="et")
            nc.vector.tensor_scalar_min(
                t[pb : pb + psz, 0:n], e[pb : pb + psz, 0:n], 1.0
            )
            nc.vector.tensor_tensor(
                dst_ap, t[pb : pb + psz, 0:n], r[pb : pb + psz, 0:n], ALU.add
            )

        # ====== conv2: x2[64, 4096] = elu1(w2 @ f2), fp8 DoubleRow ============
        f2sb = fpool.tile([128, 8 * 2048], dt.float8e4, tag="f2")
        for nb in range(8):
            nc.sync.dma_start(
                f2sb[:, nb * 2048 : (nb + 1) * 2048],
                f2d[:, nb * 2048 : (nb + 1) * 2048],
            )
        x2 = xpool.tile([64, 4096], dt.bfloat16, tag="x2")
        f3sb = fpool.tile([128, 2 * 4096], dt.bfloat16, tag="f3")
        for nb in range(8):
            ps2 = pspool.tile([128, 512], dt.float32, tag="ps", name=f"ps2_{nb}")
            for kp in range(2):
                lw = _fview(w2sb[:], kp * 128, [(64, 2), (1, 64)])
                rx = _fview(f2sb[:], nb * 2048 + kp * 1024, [(512, 2), (1, 512)])
                nc.tensor.matmul(
                    ps2[0:64, :], lw, rx, start=(kp == 0), stop=(kp == 1),
                    perf_mode=DR,
                )
            elu1(ps2, 0, 64, x2[:, nb * 512 : (nb + 1) * 512], scale=1.0 / W2SCALE)
        for ch in range(8):
            nc.sync.dma_start(
                f3sb[:, ch * 1024 : (ch + 1) * 1024],
                f3d[:, ch * 1024 : (ch + 1) * 1024],
            )

        # ====== resize: x2 [64,64,64] -> xcat[0:64, 3072:4096] ================
        rw = xpool.tile([64, 2048], dt.bfloat16, tag="rw")
        x2a = x2[:]
        rwa = rw[:]
        xc64 = xcat[0:64, :]
        # --- pass W: rw[c, h, j] = resize_w(x2[c, h, :]) ---
        out_all = _fview(rwa, 0, [(32, 64), (1, 32)])
        nc.vector.tensor_scalar_mul(out_all, _fview(x2a, 0, [(64, 64), (2, 32)]), W_IN)
        nc.vector.scalar_tensor_tensor(
            out_all, _fview(x2a, 1, [(64, 64), (2, 32)]), W_IN, out_all, ALU.mult, ALU.add
        )
        o_hi = _fview(rwa, 1, [(32, 64), (1, 31)])
        nc.vector.scalar_tensor_tensor(
            o_hi, _fview(x2a, 1, [(64, 64), (2, 31)]), W_OUT, o_hi, ALU.mult, ALU.add
        )
        o_lo = _fview(rwa, 0, [(32, 64), (1, 31)])
        nc.vector.scalar_tensor_tensor(
            o_lo, _fview(x2a, 2, [(64, 64), (2, 31)]), W_OUT, o_lo, ALU.mult, ALU.add
        )
        # boundary j=0 / j=31 (overwrite)
        oj0 = _fview(rwa, 0, [(32, 64), (1, 1)])
        nc.vector.tensor_scalar_mul(oj0, _fview(x2a, 0, [(64, 64), (1, 1)]), B3)
        nc.vector.scalar_tensor_tensor(
            oj0, _fview(x2a, 1, [(64, 64), (1, 1)]), B3, oj0, ALU.mult, ALU.add
        )
        nc.vector.scalar_tensor_tensor(
            oj0, _fview(x2a, 2, [(64, 64), (1, 1)]), B1, oj0, ALU.mult, ALU.add
        )
        oj31 = _fview(rwa, 31, [(32, 64), (1, 1)])
        nc.vector.tensor_scalar_mul(oj31, _fview(x2a, 61, [(64, 64), (1, 1)]), B1)
        nc.vector.scalar_tensor_tensor(
            oj31, _fview(x2a, 62, [(64, 64), (1, 1)]), B3, oj31, ALU.mult, ALU.add
        )
        nc.vector.scalar_tensor_tensor(
            oj31, _fview(x2a, 63, [(64, 64), (1, 1)]), B3, oj31, ALU.mult, ALU.add
        )
        # --- pass H: xcat[0:64, 3072 + 32*i + w] = resize_h(rw[c, :, w]) ---
        F2R0 = 3072
        out_all = _fview(xc64, F2R0, [(32, 32), (1, 32)])
        nc.vector.tensor_scalar_mul(out_all, _fview(rwa, 0, [(64, 32), (1, 32)]), W_IN)
        nc.vector.scalar_tensor_tensor(
            out_all, _fview(rwa, 32, [(64, 32), (1, 32)]), W_IN, out_all, ALU.mult, ALU.add
        )
        o_hi = _fview(xc64, F2R0 + 32, [(32, 31), (1, 32)])
        nc.vector.scalar_tensor_tensor(
            o_hi, _fview(rwa, 32, [(64, 31), (1, 32)]), W_OUT, o_hi, ALU.mult, ALU.add
        )
        o_lo = _fview(xc64, F2R0, [(32, 31), (1, 32)])
        nc.vector.scalar_tensor_tensor(
            o_lo, _fview(rwa, 64, [(64, 31), (1, 32)]), W_OUT, o_lo, ALU.mult, ALU.add
        )
        oi0 = _fview(xc64, F2R0, [(32, 1), (1, 32)])
        nc.vector.tensor_scalar_mul(oi0, _fview(rwa, 0, [(32, 1), (1, 32)]), B3)
        nc.vector.scalar_tensor_tensor(
            oi0, _fview(rwa, 32, [(32, 1), (1, 32)]), B3, oi0, ALU.mult, ALU.add
        )
        nc.vector.scalar_tensor_tensor(
            oi0, _fview(rwa, 64, [(32, 1), (1, 32)]), B1, oi0, ALU.mult, ALU.add
        )
        oi31 = _fview(xc64, F2R0 + 31 * 32, [(32, 1), (1, 32)])
        nc.vector.tensor_scalar_mul(oi31, _fview(rwa, 61 * 32, [(32, 1), (1, 32)]), B1)
        nc.vector.scalar_tensor_tensor(
            oi31, _fview(rwa, 62 * 32, [(32, 1), (1, 32)]), B3, oi31, ALU.mult, ALU.add
        )
        nc.vector.scalar_tensor_tensor(
            oi31, _fview(rwa, 63 * 32, [(32, 1), (1, 32)]), B3, oi31, ALU.mult, ALU.add
        )

        # ====== conv3 -> xcat[:, 0:1024] (ct0) ================================
        w3sb = wpool.tile([128, 8 * 128], dt.bfloat16, tag="w3")
        nc.sync.dma_start(w3sb[:], w3d[:])
        w4sb = wpool.tile([128, 16 * 320], dt.bfloat16, tag="w4")
        f4sb = fpool.tile([128, 2 * 8192], dt.bfloat16, tag="f4")
        w9sb = wpool.tile([128, 4 * 512], dt.bfloat16, tag="w9")
        b9sb = wpool.tile([128, 4], dt.float32, tag="b9")
        for ch in range(4):
            nc.sync.dma_start(
                w4sb[:, ch * 1280 : (ch + 1) * 1280],
                w4d[:, ch * 1280 : (ch + 1) * 1280],
            )
        for ch in range(8):
            nc.sync.dma_start(
                f4sb[:, ch * 2048 : (ch + 1) * 2048],
                f4d[:, ch * 2048 : (ch + 1) * 2048],
            )
        for kw in range(4):
            nc.sync.dma_start(
                w9sb[:, kw * 512 : (kw + 1) * 512],
                w9d[:, kw * 512 : (kw + 1) * 512],
            )
        nc.sync.dma_start(b9sb[:], b9d[:])
        for nb in range(2):
            ps3 = pspool.tile([128, 512], dt.float32, tag="ps", name=f"ps3_{nb}")
            for k in range(8):
                nc.tensor.matmul(
                    ps3[:, :],
                    w3sb[:, k * 128 : (k + 1) * 128],
                    f3sb[:, nb * 4096 + k * 512 : nb * 4096 + (k + 1) * 512],
                    start=(k == 0),
                    stop=(k == 7),
                )
            elu1(ps3, 0, 128, xcat[:, nb * 512 : (nb + 1) * 512])

        # ====== conv4 -> xcat ct1, ct2, ct3-rows-64:128 =======================
        # (w4sb/f4sb DMAs were paced from the tensor queue during conv3)
        MCH4 = [(0, 128, 0), (128, 128, 0), (256, 64, 64)]  # (moff, msz, pbase)
        for nb in range(2):
            ps4 = [
                pspool.tile([128, 512], dt.float32, tag="ps", name=f"ps4_{nb}_{i}")
                for i in range(3)
            ]
            for k in range(16):
                for mi, (moff, msz, pb) in enumerate(MCH4):
                    nc.tensor.matmul(
                        ps4[mi][pb : pb + msz, :],
                        w4sb[:, k * 320 + moff : k * 320 + moff + msz],
                        f4sb[:, nb * 8192 + k * 512 : nb * 8192 + (k + 1) * 512],
                        start=(k == 0),
                        stop=(k == 15),
                        tile_position=(0, pb),
                    )
            for mi, (moff, msz, pb) in enumerate(MCH4):
                dst = xcat[pb : pb + msz, (1 + mi) * 1024 + nb * 512 : (1 + mi) * 1024 + (nb + 1) * 512]
                elu1(ps4[mi], pb, msz, dst)

        # ====== conv9 -> xbuf (stores x9+1; b9 bias removes the xcat +1) ======
        for m in range(4):
            for n in range(2):
                ps9 = pspool.tile([128, 512], dt.float32, tag="ps", name=f"ps9_{m}_{n}")
                for k in range(4):
                    nc.tensor.matmul(
                        ps9[:, :],
                        w9sb[:, k * 512 + m * 128 : k * 512 + (m + 1) * 128],
                        xcat[:, k * 1024 + n * 512 : k * 1024 + (n + 1) * 512],
                        start=(k == 0),
                        stop=(k == 3),
                    )
                elu1(
                    ps9, 0, 128,
                    xbuf[:, m * 1024 + n * 512 : m * 1024 + (n + 1) * 512],
                    bias=b9sb[:, m : m + 1],
                )
            # xodd[p, j] = xbuf[p, j+1] for this ct (4B-aligned odd-dx views)
            nc.vector.tensor_copy(
                xodd[:, m * 1024 : m * 1024 + 1022],
                xbuf[:, m * 1024 + 1 : m * 1024 + 1023],
            )

        # ====== S rows: s128[p, :] = -(sum_c xbuf)/256, replicated on all 128
        # partitions so a K=128 ones-matmul contributes exactly -S/2 (and the
        # elu +1 shift cancels against the shift carried by the maxes).
        s128 = xpool.tile([128, 1024], dt.bfloat16, tag="s128")
        s128o = xpool.tile([128, 1024], dt.bfloat16, tag="s128o")
        s128t = xpool.tile([128, 512], dt.bfloat16, tag="s128t")
        for h in range(2):
            ps_s = pspool.tile([128, 512], dt.float32, tag="pss", name=f"ps_s{h}", bufs=1)
            for ct in range(4):
                nc.tensor.matmul(
                    ps_s[:, :],
                    ones128[:, 0:128],
                    xbuf[:, ct * 1024 + h * 512 : ct * 1024 + (h + 1) * 512],
                    start=(ct == 0),
                    stop=(ct == 3),
                )
            # rows 0:64 = bf16(-S/128) (hi); rows 64:128 = bf16 residual (lo);
            # the K=128 ones-sum then yields 64*(hi+lo) = -S/2 to ~fp32 accuracy
            hs = h * 512
            nc.scalar.activation(
                s128[0:64, hs : hs + 512], ps_s[0:64, :], ACTF.Copy,
                scale=-1.0 / 128.0,
            )
            nc.scalar.activation(
                s128t[64:128, 0:512], ps_s[64:128, :], ACTF.Copy,
                scale=-1.0 / 128.0,
            )
            nc.vector.scalar_tensor_tensor(
                s128[64:128, hs : hs + 512], ps_s[64:128, :], -1.0 / 128.0,
                s128t[64:128, 0:512], ALU.mult, ALU.subtract,
            )
        # odd-shifted copy for odd-dx to-views (4B alignment)
        nc.vector.tensor_copy(s128o[:, 0:1022], s128[:, 1:1023])

        # ====== affinity ======================================================
        xba = xbuf[:]
        xoa = xodd[:]
        s1a = s128[:]
        s1oa = s128o[:]

        # DVE maxes per (group, ct): mt[g][ct] = max(ft, ff) in group layout
        mtiles = {}
        for g, (dy, dxs) in enumerate(GROUPS):
            ndx = len(dxs)
            G = ndx * NPOS
            odd = dxs[0] % 2 != 0
            for ct in range(4):
                ctb = ct * 1024
                ff = _fview(xba, ctb + 4, [(0, ndx), (32, CH), (1, CW)])
                if odd:
                    ft = _fview(
                        xoa, ctb + 32 * dy + 4 + dxs[0] - 1,
                        [(2, ndx), (32, CH), (1, CW)],
                    )
                else:
                    ft = _fview(
                        xba, ctb + 32 * dy + 4 + dxs[0],
                        [(2, ndx), (32, CH), (1, CW)],
                    )
                mt = mpool.tile([128, G], dt.bfloat16, tag="m", name=f"m_{g}_{ct}")
                mv = _fview(mt[:], 0, [(NPOS, ndx), (CW, CH), (1, CW)])
                nc.vector.tensor_tensor(mv, ft, ff, ALU.max)
                mtiles[(g, ct)] = mt

        # PE channel sums + corrections; 4 slots per PSUM bank; batched extract
        for b in range(N_BANKS):
            psb = psb_pool.tile([128, CHUNK], dt.float32, tag="psb", name=f"psb_{b}")
            for q in range(4):
                s = 4 * b + q
                g, c0, cs = _SLOTS[s]
                row = 32 * q
                for ct in range(4):
                    nc.tensor.matmul(
                        psb[row : row + 1, 0:cs],
                        ones[:, 0:1],
                        mtiles[(g, ct)][:, c0 : c0 + cs],
                        start=(ct == 0),
                        stop=False,
                        tile_position=(0, row),
                    )
            for q in range(4):
                s = 4 * b + q
                g, c0, cs = _SLOTS[s]
                dy, dxs = GROUPS[g]
                row = 32 * q
                # corrections: -(S_from + S_to)/2 via K=128 matmuls on the
                # replicated -S/256 rows (same ones stationary as the sums)
                dxi, h2 = (c0 // NPOS), (c0 % NPOS) // CHUNK
                from_v = _fview(s1a, 4 + 32 * 14 * h2, [(32, 14), (1, CW)])
                dx = dxs[dxi]
                if dx % 2 != 0:
                    to_v = _fview(
                        s1oa, 4 + 32 * (dy + 14 * h2) + dx - 1, [(32, 14), (1, CW)]
                    )
                else:
                    to_v = _fview(
                        s1a, 4 + 32 * (dy + 14 * h2) + dx, [(32, 14), (1, CW)]
                    )
                nc.tensor.matmul(
                    psb[row : row + 1, 0:cs], ones[:, 0:1], from_v,
                    start=False, stop=False, tile_position=(0, row),
                )
                nc.tensor.matmul(
                    psb[row : row + 1, 0:cs], ones[:, 0:1], to_v,
                    start=False, stop=True, tile_position=(0, row),
                )
            # batched extraction: aff = exp(-psum/256) on the whole bank
            ex = opool.tile([128, CHUNK], dt.float32, tag="ex", name=f"ex_{b}")
            nc.scalar.activation(ex[:], psb[:], ACTF.Exp, scale=-1.0 / 256.0)
            nc.sync.dma_start(
                outd[4 * b : 4 * b + 4, :], _prow_view(ex[:], 32, 4, CHUNK)
            )

    nc.compile()
    return nc


# ------------------------------ host wrapper ---------------------------------

_NC_CACHE = None
LAST_EXEC_NS = None
LAST_MEAN_EXEC_NS = None


def _get_nc():
    global _NC_CACHE
    if _NC_CACHE is None:
        _NC_CACHE = build_nc()
    return _NC_CACHE


def _nblock(a, n_nb, n_kt):
    """[C, S] -> [128, nb*(n_kt*512) + kt*512 + c] n-block-major layout.

    a has C = 128*n_kt channels, S = n_nb*512 positions per channel.
    """
    c, s = a.shape
    assert c == 128 * n_kt and s == n_nb * 512
    # a[kt*128 + p, nb*512 + c] -> out[p, nb*n_kt*512 + kt*512 + c]
    v = a.reshape(n_kt, 128, n_nb, 512)
    return np.ascontiguousarray(v.transpose(1, 2, 0, 3).reshape(128, n_nb * n_kt * 512))


def _prep_inputs(f2_in, f3_in, f4_in, w2, w3, w4, w9):
    """Shard/tile/cast on host. Returns per-core input maps."""

    def ctile(a, k):  # [C, S] -> [128, k*S] with c-tile t at cols [t*S, (t+1)*S)
        c, s = a.shape
        assert c == 128 * k
        return np.ascontiguousarray(
            a.reshape(k, 128, s).transpose(1, 0, 2).reshape(128, k * s)
        )

    # w2: scale, fp8, ktile-pair layout [p, kp*128 + j*64 + m]
    w2s = (np.asarray(w2, np.float32).T * W2SCALE).astype(FP8)   # [512, 64]
    w2t = np.ascontiguousarray(
        w2s.reshape(2, 2, 128, 64).transpose(2, 0, 1, 3).reshape(128, 256)
    )
    w3t = ctile(np.asarray(w3, np.float32).T.astype(BF16), 8)      # [1024,128]
    w4t = ctile(np.asarray(w4, np.float32).T.astype(BF16), 16)     # [2048,320]
    w9p = np.asarray(w9, np.float32)[:, XCAT_PERM].T               # [512 in, 512 out]
    w9t = ctile(w9p.astype(BF16), 4)
    # b9[p, m] = -sum_k w9p[k, m*128+p] (removes the +1 carried by xcat)
    b9f = -np.asarray(w9p, np.float32).astype(BF16).astype(np.float32).sum(axis=0)
    b9 = np.ascontiguousarray(b9f.reshape(4, 128).T.astype(np.float32))

    f2 = np.asarray(f2_in, np.float32).reshape(B, 512, 4096).astype(FP8)
    f3 = np.asarray(f3_in, np.float32).reshape(B, 1024, 1024).astype(BF16)
    f4 = np.asarray(f4_in, np.float32).reshape(B, 2048, 1024).astype(BF16)

    in_maps = []
    for b in range(B):
        in_maps.append(
            {
                "f2": _nblock(f2[b], 8, 4),
                "f3": _nblock(f3[b], 2, 8),
                "f4": _nblock(f4[b], 2, 16),
                "w2t": w2t,
                "w3t": w3t,
                "w4t": w4t,
                "w9t": w9t,
                "b9": b9,
            }
        )
    return in_maps


def _install_trace_hooks():
    import types

    if "antenv.axon_hooks" not in sys.modules:
        mod = types.ModuleType("antenv.axon_hooks")
        _HOOK = [None]
        mod.set_axon_ntff_profile_hook = lambda h: _HOOK.__setitem__(0, h)
        mod.get_axon_ntff_profile_hook = lambda: _HOOK[0]
        sys.modules["antenv.axon_hooks"] = mod
        from trn_agent_boot.trn_boot import _ntff_profile_via_ctypes

        mod.set_axon_ntff_profile_hook(
            _ntff_profile_via_ctypes("/opt/axon/libaxon_pjrt.so")
        )
    import concourse.bass_utils as bass_utils

    bass_utils.upload_artifacts = lambda tmpdir: f"local:{tmpdir}"


def kernel(f2_in, f3_in, f4_in, w2, w3, w4, w9, _trace=False, _tmpdir=None):
    global LAST_EXEC_NS, LAST_MEAN_EXEC_NS
    from concourse.bass_utils import run_bass_kernel_spmd

    if _trace:
        _install_trace_hooks()

    nc = _get_nc()
    in_maps = _prep_inputs(f2_in, f3_in, f4_in, w2, w3, w4, w9)
    res = run_bass_kernel_spmd(
        nc, in_maps, list(range(N_CORES)), trace=_trace, tmpdir=_tmpdir
    )
    LAST_EXEC_NS = res.exec_time_ns
    LAST_MEAN_EXEC_NS = res.mean_exec_time_ns

    out = np.empty((B, 34, NPOS), np.float32)
    for b in range(B):
        flat = res.results[b]["aff"].reshape(-1)
        out[b] = flat[_FLAT_IDX]
    return out
